# revision 3
# baseline (speedup 1.0000x reference)
"""MHA kernel for trn2, 8 NeuronCores, head-sharded (2 heads/core).

Per core c (heads 2c, 2c+1):
  qT/kT = (w_{q,k} shard).T @ x.T  -> [128, T] bf16 (rows 0:64 head a, 64:128 head b)
  v     = x @ w_v shard            -> [T, 128]
  per q-tile (512) x key-block (128):
     S^T = kT_blk.T @ qT   (row-tiled pair, K=64 per head, one [128,1024] psum)
     eS  = exp(S^T + BIAS) -- engine/dtype per key-block class:
           A16: ACT exp -> bf16;  D16: DVE Schraudolph (affine->int16 bits
           viewed as bf16);  A8: ACT exp -> fp8e4 (global BIAS keeps range)
     y'[65,512] += Vp[kb].T @ eS   (Vp = [V | ones]; row 64 = Z = sum exp)
           A8 blocks: fp8 DoubleRow matmul, stationary = (V_hi, V_lo) pair,
           moving = eS8 broadcast to both k-tiles (full-precision V, 2x rate)
  normalization on device: Z -> partition 0 via sbuf DMA, reciprocal,
  gpsimd partition_broadcast, DVE multiply -> yns [128,512] bf16
  fused out-proj (K=128 = both heads): outT[:, qtile] += wo.T @ yns
Host: sums the 8 cores' outT partials.
"""

import numpy as np
import ml_dtypes

import concourse.bacc as bacc
import concourse.mybir as mybir
from concourse.tile import TileContext
from concourse.bass_utils import run_bass_kernel_spmd

BF16 = ml_dtypes.bfloat16
F32 = mybir.dt.float32
BF = mybir.dt.bfloat16
I16 = mybir.dt.int16
FP8 = mybir.dt.float8e4
EXP = mybir.ActivationFunctionType.Exp
MULT = mybir.AluOpType.mult
ADD = mybir.AluOpType.add
SUB = mybir.AluOpType.subtract
DR = mybir.MatmulPerfMode.DoubleRow

B, T, C = 1, 4096, 1024
H, D = 16, 64
NCORES = 8
P = 128
CB = C // P          # 8 contraction blocks
KB = T // P          # 32 key blocks
QTS = T // 512       # 8 q tiles

BIAS = -3.75
A16C = 128.0 / np.log(2.0)
B16 = 16248.65 + A16C * BIAS

# per key-block class: A8 = fp8 PV (fastest PE), D16 = DVE exp (offloads ACT),
# A16 = plain ACT exp -> bf16 (most accurate)
N_A8 = 20
N_D16 = 11


def _mkcls():
    cls = ["A16"] * KB
    if N_A8:
        for i in np.linspace(0, KB - 1, N_A8).round().astype(int):
            cls[i] = "A8"
    rest = [i for i in range(KB) if cls[i] == "A16"]
    if N_D16:
        for i in np.array(rest)[np.linspace(0, len(rest) - 1, N_D16).round().astype(int)]:
            cls[i] = "D16"
    return cls


CLS = _mkcls()

_cached = None


def build_bass():
    global _cached
    if _cached is not None:
        return _cached

    nc = bacc.Bacc("TRN2", target_bir_lowering=False, name="mha_v2")

    xT = nc.dram_tensor("xT", (C, T), BF, kind="ExternalInput")
    wq = nc.dram_tensor("wq", (C, P), BF, kind="ExternalInput")
    wk = nc.dram_tensor("wk", (C, P), BF, kind="ExternalInput")
    wv = nc.dram_tensor("wv", (C, P), BF, kind="ExternalInput")
    wo = nc.dram_tensor("wo", (P, C), BF, kind="ExternalInput")
    outT = nc.dram_tensor("outT", (C, T), BF, kind="ExternalOutput")

    with TileContext(nc) as tc:
        with (
            tc.tile_pool(name="const", bufs=1) as const,
            tc.tile_pool(name="work", bufs=3) as work,
            tc.tile_pool(name="psS", bufs=2, space="PSUM") as psS,
            tc.tile_pool(name="psY", bufs=1, space="PSUM") as psY,
            tc.tile_pool(name="psO", bufs=2, space="PSUM") as psO,
        ):
            # ---- load inputs (weights on gpsimd queue, x on sync queue) ----
            wqs = const.tile([P, CB, P], BF)
            nc.gpsimd.dma_start(wqs[:], wq[:, :].rearrange("(cb p) f -> p cb f", p=P))
            wks = const.tile([P, CB, P], BF)
            nc.gpsimd.dma_start(wks[:], wk[:, :].rearrange("(cb p) f -> p cb f", p=P))
            wvs = const.tile([P, CB, P], BF)
            nc.gpsimd.dma_start(wvs[:], wv[:, :].rearrange("(cb p) f -> p cb f", p=P))
            wos = const.tile([P, C], BF)
            nc.gpsimd.dma_start(wos[:], wo[:, :])
            xTs = const.tile([P, CB, T], BF)
            xT_r = xT[:, :].rearrange("(cb p) t -> p cb t", p=P)
            for tt in range(QTS):
                nc.sync.dma_start(xTs[:, :, tt * 512:(tt + 1) * 512],
                                  xT_r[:, :, tt * 512:(tt + 1) * 512])

            bias_t = const.tile([P, 1], F32)
            nc.vector.memset(bias_t[:], BIAS)

            # ---- warm up the PE clock (HAM) during the input DMA wait ----
            warm = const.tile([P, 512], BF)
            nc.vector.memset(warm[:], 0.0)
            for _ in range(30):
                pw = psO.tile([P, 512], F32, tag="po", name="pw")
                nc.tensor.matmul(pw[:, :], warm[:, 0:P], warm[:, :],
                                 start=True, stop=True)

            qTs = const.tile([P, T], BF)
            kTs = const.tile([P, T], BF)
            eS_store = {}

            def emit_s_exp(qt, kb):
                q0 = qt * 512
                k0 = kb * P
                s = psS.tile([P, 1024], F32, tag="s", name="s")
                nc.tensor.matmul(s[:, 0:512], kTs[0:64, k0:k0 + P],
                                 qTs[0:64, q0:q0 + 512], start=True, stop=True)
                nc.tensor.matmul(s[:, 512:1024], kTs[64:128, k0:k0 + P],
                                 qTs[64:128, q0:q0 + 512], start=True, stop=True)
                cl = CLS[kb]
                if cl == "A8":
                    eS = work.tile([P, 1024], FP8, tag="es8", bufs=17)
                    nc.scalar.activation(eS[:], s[:], EXP, bias=bias_t[:])
                elif cl == "D16":
                    eS = work.tile([P, 1024], BF, tag="esb", bufs=17)
                    nc.vector.tensor_scalar(eS[:].bitcast(I16), s[:], A16C, B16,
                                            op0=MULT, op1=ADD)
                else:
                    eS = work.tile([P, 1024], BF, tag="esb", bufs=17)
                    nc.scalar.activation(eS[:], s[:], EXP, bias=bias_t[:])
                eS_store[(qt, kb)] = eS

            LAG = 14

            # Vp for bf16 blocks: [V | ones]; for fp8 blocks: (hi, lo) pairs
            VpB0 = const.tile([P, KB, 65], BF)
            VpB1 = const.tile([P, KB, 65], BF)
            nc.vector.memset(VpB0[:, :, 64:65], 1.0)
            nc.vector.memset(VpB1[:, :, 64:65], 1.0)
            Vp80 = const.tile([P, KB, 2, 80], FP8)
            Vp81 = const.tile([P, KB, 2, 80], FP8)
            nc.vector.memset(Vp80[:, :, 0, 64:65], 1.0)
            nc.vector.memset(Vp80[:, :, 1, 64:65], 0.0)
            nc.vector.memset(Vp81[:, :, 0, 64:65], 1.0)
            nc.vector.memset(Vp81[:, :, 1, 64:65], 0.0)

            for tt in range(QTS):
                pq = psS.tile([P, 1024], F32, tag="s")
                for cb in range(CB):
                    nc.tensor.matmul(
                        pq[:, 0:512], wks[:, cb, :], xTs[:, cb, tt * 512:(tt + 1) * 512],
                        start=(cb == 0), stop=(cb == CB - 1))
                if tt == 0:
                    for cb in range(CB):
                        nc.tensor.matmul(
                            pq[:, 512:1024], wqs[:, cb, :], xTs[:, cb, 0:512],
                            start=(cb == 0), stop=(cb == CB - 1))
                nc.scalar.copy(kTs[:, tt * 512:(tt + 1) * 512], pq[:, 0:512])
                if tt == 0:
                    nc.vector.tensor_copy(qTs[:, 0:512], pq[:, 512:1024])
                for tb in range(tt * 4, tt * 4 + 4):
                    pv = psO.tile([P, 512], F32, tag="po", name="pv")
                    for cb in range(CB):
                        nc.tensor.matmul(
                            pv[:, 0:P], xTs[:, cb, tb * P:(tb + 1) * P], wvs[:, cb, :],
                            start=(cb == 0), stop=(cb == CB - 1))
                    if CLS[tb] == "A8":
                        nc.scalar.copy(Vp80[:, tb, 0, 0:64], pv[:, 0:64])
                        nc.vector.tensor_tensor(
                            Vp80[:, tb, 1, 0:64], pv[:, 0:64], Vp80[:, tb, 0, 0:64],
                            op=SUB)
                        nc.scalar.copy(Vp81[:, tb, 0, 0:64], pv[:, 64:128])
                        nc.vector.tensor_tensor(
                            Vp81[:, tb, 1, 0:64], pv[:, 64:128], Vp81[:, tb, 0, 0:64],
                            op=SUB)
                    else:
                        nc.scalar.copy(VpB0[:, tb, 0:64], pv[:, 0:64])
                        nc.vector.tensor_copy(VpB1[:, tb, 0:64], pv[:, 64:128])
                # pre-emit S^T+exp pairs of q-tile 0 so ACT starts early
                if tt >= 1:
                    for kb_pre in range((tt - 1) * 2, (tt - 1) * 2 + 2):
                        emit_s_exp(0, kb_pre)

            def emit_qt_proj(tokt):
                pqd = psO.tile([P, 512], F32, tag="po", name="pqd")
                for cb in range(CB):
                    nc.tensor.matmul(
                        pqd[:, :], wqs[:, cb, :], xTs[:, cb, tokt * 512:(tokt + 1) * 512],
                        start=(cb == 0), stop=(cb == CB - 1))
                nc.vector.tensor_copy(qTs[:, tokt * 512:(tokt + 1) * 512], pqd[:])

            # ---- fused out-projection: both heads in one K=128 matmul ----
            def emit_outproj(dep, fb, flush=False):
                yns, q0 = dep
                po = psO.tile([P, 512], F32, tag="po", name="po")
                nc.tensor.matmul(po[:, :], wos[:, fb * P:(fb + 1) * P], yns[:, :],
                                 start=True, stop=True)
                oc = work.tile([P, 512], BF, tag="oc", bufs=8)
                if flush and fb % 2 == 1:
                    nc.scalar.copy(oc[:], po[:])
                else:
                    nc.vector.tensor_copy(oc[:], po[:])
                if fb % 2 == 0:
                    nc.gpsimd.dma_start(outT[fb * P:(fb + 1) * P, q0:q0 + 512], oc[:])
                else:
                    nc.sync.dma_start(outT[fb * P:(fb + 1) * P, q0:q0 + 512], oc[:])

            OP_SLOTS = {8: 0, 11: 1, 14: 2, 17: 3, 20: 4, 23: 5, 26: 6, 29: 7}

            pending = None
            for qt in range(QTS):
                q0 = qt * 512
                y0 = psY.tile([65, 512], F32, tag="y0")
                y1 = psY.tile([65, 512], F32, tag="y1")
                for kb in range(KB):
                    tgt = qt * KB + kb + LAG
                    if tgt < QTS * KB:
                        emit_s_exp(tgt // KB, tgt % KB)
                    eS = eS_store.pop((qt, kb))
                    if CLS[kb] == "A8":
                        mov0 = eS[:, 0:512].unsqueeze(1).broadcast_to([P, 2, 512])
                        mov1 = eS[:, 512:1024].unsqueeze(1).broadcast_to([P, 2, 512])
                        nc.tensor.matmul(y0[:, :], Vp80[:, kb, :, 0:65], mov0,
                                         start=(kb == 0), stop=(kb == KB - 1),
                                         perf_mode=DR)
                        nc.tensor.matmul(y1[:, :], Vp81[:, kb, :, 0:65], mov1,
                                         start=(kb == 0), stop=(kb == KB - 1),
                                         perf_mode=DR)
                    else:
                        nc.tensor.matmul(y0[:, :], VpB0[:, kb, :], eS[:, 0:512],
                                         start=(kb == 0), stop=(kb == KB - 1))
                        nc.tensor.matmul(y1[:, :], VpB1[:, kb, :], eS[:, 512:1024],
                                         start=(kb == 0), stop=(kb == KB - 1))
                    if pending is not None and kb in OP_SLOTS:
                        emit_outproj(pending, OP_SLOTS[kb])
                    if kb == 3 and qt + 1 < QTS:
                        emit_qt_proj(qt + 1)

                # ---- normalize by Z on device, assemble yns [128, 512] bf16 ----
                ztmp = work.tile([65, 1024], F32, tag="ztmp", bufs=2)
                nc.vector.tensor_copy(ztmp[64:65, 0:512], y0[64:65, :])
                nc.vector.tensor_copy(ztmp[64:65, 512:1024], y1[64:65, :])
                zrow = work.tile([1, 1024], F32, tag="zrow", bufs=2)
                nc.sync.dma_start(zrow[0:1, :], ztmp[64:65, :])
                zrec = work.tile([1, 1024], F32, tag="zrec", bufs=2)
                nc.vector.reciprocal(zrec[:], zrow[:])
                zb = work.tile([64, 1024], F32, tag="zb", bufs=2)
                nc.gpsimd.partition_broadcast(zb[:, 0:512], zrec[0:1, 0:512])
                nc.gpsimd.partition_broadcast(zb[:, 512:1024], zrec[0:1, 512:1024])
                yns = work.tile([P, 512], BF, tag="yns", bufs=2)
                nc.vector.tensor_tensor(yns[0:64, :], y0[0:64, :], zb[:, 0:512],
                                        op=MULT)
                y1t = work.tile([64, 512], BF, tag="y1t", bufs=2)
                nc.vector.tensor_tensor(y1t[:], y1[0:64, :], zb[:, 512:1024],
                                        op=MULT)
                nc.gpsimd.dma_start(yns[64:128, :], y1t[:])
                pending = (yns, q0)

            for fb in range(CB):
                emit_outproj(pending, fb, flush=True)

    nc.compile()
    _cached = nc
    return nc


def make_in_maps(x, w_qkv, w_out):
    """x [1,T,C] f32, w_qkv [C, 3C] f32, w_out [C, C] f32 -> per-core input dicts."""
    x = np.asarray(x, dtype=np.float32)
    w_qkv = np.asarray(w_qkv, dtype=np.float32)
    w_out = np.asarray(w_out, dtype=np.float32)
    scale = 1.0 / np.sqrt(np.float32(D))
    xT = np.ascontiguousarray(x.reshape(T, C).T).astype(BF16)  # [C, T]
    in_maps = []
    for c in range(NCORES):
        cols = slice(P * c, P * (c + 1))
        wq = np.ascontiguousarray(w_qkv[:, 0:C][:, cols] * scale).astype(BF16)
        wk = np.ascontiguousarray(w_qkv[:, C:2 * C][:, cols]).astype(BF16)
        wv = np.ascontiguousarray(w_qkv[:, 2 * C:3 * C][:, cols]).astype(BF16)
        wo = np.ascontiguousarray(w_out[P * c:P * (c + 1), :]).astype(BF16)
        in_maps.append({"xT": xT, "wq": wq, "wk": wk, "wv": wv, "wo": wo})
    return in_maps


def run(x, w_qkv, w_out, trace=False):
    nc = build_bass()
    in_maps = make_in_maps(x, w_qkv, w_out)
    res = run_bass_kernel_spmd(nc, in_maps, core_ids=list(range(NCORES)), trace=trace)
    acc = np.zeros((C, T), dtype=np.float32)
    for r in res.results:
        acc += r["outT"].astype(np.float32)
    out = np.ascontiguousarray(acc.T).reshape(B, T, C)
    return out, res


def kernel(x, w_qkv, w_out):
    out, _ = run(x, w_qkv, w_out, trace=False)
    return out


# revision 7
# speedup vs baseline: 1.1508x; 1.1508x over previous
"""MHA kernel for trn2, 8 NeuronCores, head-sharded (2 heads/core).

Per core c (heads 2c, 2c+1):
  qT/kT = (w_{q,k} shard).T @ x.T  -> [128, T] bf16 (rows 0:64 head a, 64:128 head b)
  v     = x @ w_v shard            -> [T, 128]
  per q-tile (512) x key-block (128):
     S^T = kT_blk.T @ qT   (row-tiled pair, K=64 per head, one [128,1024] psum)
     eS  = exp(S^T + BIAS) -- engine/dtype per key-block class:
           A16: ACT exp -> bf16;  D16: DVE Schraudolph (affine->int16 bits
           viewed as bf16);  A8: ACT exp -> fp8e4 (global BIAS keeps range)
     y'[65,512] += Vp[kb].T @ eS   (Vp = [V | ones]; row 64 = Z = sum exp)
           A8 blocks: fp8 DoubleRow matmul, stationary = (V_hi, V_lo) pair,
           moving = eS8 broadcast to both k-tiles (full-precision V, 2x rate)
  normalization on device: Z -> partition 0 via sbuf DMA, reciprocal,
  gpsimd partition_broadcast, DVE multiply -> yns [128,512] bf16
  fused out-proj (K=128 = both heads): outT[:, qtile] += wo.T @ yns
Host: sums the 8 cores' outT partials.
"""

import numpy as np
import ml_dtypes

import concourse.bacc as bacc
import concourse.mybir as mybir
from concourse.tile import TileContext
from concourse.bass_utils import run_bass_kernel_spmd

BF16 = ml_dtypes.bfloat16
F32 = mybir.dt.float32
BF = mybir.dt.bfloat16
I16 = mybir.dt.int16
FP8 = mybir.dt.float8e4
EXP = mybir.ActivationFunctionType.Exp
MULT = mybir.AluOpType.mult
ADD = mybir.AluOpType.add
SUB = mybir.AluOpType.subtract
DR = mybir.MatmulPerfMode.DoubleRow

B, T, C = 1, 4096, 1024
H, D = 16, 64
NCORES = 8
P = 128
CB = C // P          # 8 contraction blocks
KB = T // P          # 32 key blocks
QTS = T // 512       # 8 q tiles

BIAS = -3.75
A16C = 128.0 / np.log(2.0)
B16 = 16248.65 + A16C * BIAS

# per key-block class: A8 = fp8 PV (fastest PE), D16 = DVE exp (offloads ACT),
# A16 = plain ACT exp -> bf16 (most accurate)
N_A8 = 20
N_D16 = 12


def _mkcls():
    cls = ["A16"] * KB
    if N_A8:
        for i in np.linspace(0, KB - 1, N_A8).round().astype(int):
            cls[i] = "A8"
    rest = [i for i in range(KB) if cls[i] == "A16"]
    if N_D16:
        for i in np.array(rest)[np.linspace(0, len(rest) - 1, N_D16).round().astype(int)]:
            cls[i] = "D16"
    return cls


CLS = _mkcls()

_cached = None


def build_bass():
    global _cached
    if _cached is not None:
        return _cached

    nc = bacc.Bacc("TRN2", target_bir_lowering=False, name="mha_v2")

    xT = nc.dram_tensor("xT", (C, T), BF, kind="ExternalInput")
    wq = nc.dram_tensor("wq", (C, P), BF, kind="ExternalInput")
    wk = nc.dram_tensor("wk", (C, P), BF, kind="ExternalInput")
    wv = nc.dram_tensor("wv", (C, P), BF, kind="ExternalInput")
    wo = nc.dram_tensor("wo", (P, C), BF, kind="ExternalInput")
    outT = nc.dram_tensor("outT", (C, T), BF, kind="ExternalOutput")

    with TileContext(nc) as tc:
        with (
            tc.tile_pool(name="const", bufs=1) as const,
            tc.tile_pool(name="work", bufs=3) as work,
            tc.tile_pool(name="psS", bufs=2, space="PSUM") as psS,
            tc.tile_pool(name="psY", bufs=1, space="PSUM") as psY,
            tc.tile_pool(name="psO", bufs=2, space="PSUM") as psO,
        ):
            # ---- load inputs (weights on gpsimd queue, x on sync queue) ----
            wqs = const.tile([P, CB, P], BF)
            nc.gpsimd.dma_start(wqs[:], wq[:, :].rearrange("(cb p) f -> p cb f", p=P))
            wks = const.tile([P, CB, P], BF)
            nc.gpsimd.dma_start(wks[:], wk[:, :].rearrange("(cb p) f -> p cb f", p=P))
            wvs = const.tile([P, CB, P], BF)
            nc.gpsimd.dma_start(wvs[:], wv[:, :].rearrange("(cb p) f -> p cb f", p=P))
            wos = const.tile([P, C], BF)
            nc.gpsimd.dma_start(wos[:], wo[:, :])
            xTs = const.tile([P, CB, T], BF)
            xT_r = xT[:, :].rearrange("(cb p) t -> p cb t", p=P)
            for tt in range(QTS):
                nc.sync.dma_start(xTs[:, :, tt * 512:(tt + 1) * 512],
                                  xT_r[:, :, tt * 512:(tt + 1) * 512])

            bias_t = const.tile([P, 1], F32)
            nc.vector.memset(bias_t[:], BIAS)

            # ---- warm up the PE clock (HAM) during the input DMA wait ----
            warm = const.tile([P, 512], BF)
            nc.vector.memset(warm[:], 0.0)
            for _ in range(30):
                pw = psO.tile([P, 512], F32, tag="po", name="pw")
                nc.tensor.matmul(pw[:, :], warm[:, 0:P], warm[:, :],
                                 start=True, stop=True)

            qTs = const.tile([P, T], BF)
            kTs = const.tile([P, T], BF)
            eS_store = {}

            def emit_s_exp(qt, kb):
                q0 = qt * 512
                k0 = kb * P
                s = psS.tile([P, 1024], F32, tag="s", name="s")
                nc.tensor.matmul(s[:, 0:512], kTs[0:64, k0:k0 + P],
                                 qTs[0:64, q0:q0 + 512], start=True, stop=True)
                nc.tensor.matmul(s[:, 512:1024], kTs[64:128, k0:k0 + P],
                                 qTs[64:128, q0:q0 + 512], start=True, stop=True)
                cl = CLS[kb]
                if cl == "A8":
                    eS = work.tile([P, 1024], FP8, tag="es8", bufs=17)
                    nc.scalar.activation(eS[:], s[:], EXP, bias=bias_t[:])
                elif cl == "D16":
                    eS = work.tile([P, 1024], BF, tag="esb", bufs=17)
                    nc.vector.tensor_scalar(eS[:].bitcast(I16), s[:], A16C, B16,
                                            op0=MULT, op1=ADD)
                else:
                    eS = work.tile([P, 1024], BF, tag="esb", bufs=17)
                    nc.scalar.activation(eS[:], s[:], EXP, bias=bias_t[:])
                eS_store[(qt, kb)] = eS

            LAG = 14

            # Vp for bf16 blocks: [V | ones]; for fp8 blocks: (hi, lo) pairs
            VpB0 = const.tile([P, KB, 65], BF)
            VpB1 = const.tile([P, KB, 65], BF)
            nc.vector.memset(VpB0[:, :, 64:65], 1.0)
            nc.vector.memset(VpB1[:, :, 64:65], 1.0)
            Vp80 = const.tile([P, KB, 2, 80], FP8)
            Vp81 = const.tile([P, KB, 2, 80], FP8)
            nc.vector.memset(Vp80[:, :, 0, 64:65], 1.0)
            nc.vector.memset(Vp80[:, :, 1, 64:65], 0.0)
            nc.vector.memset(Vp81[:, :, 0, 64:65], 1.0)
            nc.vector.memset(Vp81[:, :, 1, 64:65], 0.0)

            for tt in range(QTS):
                pq = psS.tile([P, 1024], F32, tag="s")
                for cb in range(CB):
                    nc.tensor.matmul(
                        pq[:, 0:512], wks[:, cb, :], xTs[:, cb, tt * 512:(tt + 1) * 512],
                        start=(cb == 0), stop=(cb == CB - 1))
                if tt == 0:
                    for cb in range(CB):
                        nc.tensor.matmul(
                            pq[:, 512:1024], wqs[:, cb, :], xTs[:, cb, 0:512],
                            start=(cb == 0), stop=(cb == CB - 1))
                nc.scalar.copy(kTs[:, tt * 512:(tt + 1) * 512], pq[:, 0:512])
                if tt == 0:
                    nc.vector.tensor_copy(qTs[:, 0:512], pq[:, 512:1024])
                for tb in range(tt * 4, tt * 4 + 4):
                    pv = psO.tile([P, 512], F32, tag="po", name="pv")
                    for cb in range(CB):
                        nc.tensor.matmul(
                            pv[:, 0:P], xTs[:, cb, tb * P:(tb + 1) * P], wvs[:, cb, :],
                            start=(cb == 0), stop=(cb == CB - 1))
                    if CLS[tb] == "A8":
                        nc.scalar.copy(Vp80[:, tb, 0, 0:64], pv[:, 0:64])
                        nc.vector.tensor_tensor(
                            Vp80[:, tb, 1, 0:64], pv[:, 0:64], Vp80[:, tb, 0, 0:64],
                            op=SUB)
                        nc.scalar.copy(Vp81[:, tb, 0, 0:64], pv[:, 64:128])
                        nc.vector.tensor_tensor(
                            Vp81[:, tb, 1, 0:64], pv[:, 64:128], Vp81[:, tb, 0, 0:64],
                            op=SUB)
                    else:
                        nc.scalar.copy(VpB0[:, tb, 0:64], pv[:, 0:64])
                        nc.vector.tensor_copy(VpB1[:, tb, 0:64], pv[:, 64:128])
                # pre-emit S^T+exp pairs of q-tile 0 so ACT starts early
                if tt >= 1:
                    for kb_pre in range((tt - 1) * 2, (tt - 1) * 2 + 2):
                        emit_s_exp(0, kb_pre)

            def emit_qt_proj(tokt):
                pqd = psO.tile([P, 512], F32, tag="po", name="pqd")
                for cb in range(CB):
                    nc.tensor.matmul(
                        pqd[:, :], wqs[:, cb, :], xTs[:, cb, tokt * 512:(tokt + 1) * 512],
                        start=(cb == 0), stop=(cb == CB - 1))
                nc.vector.tensor_copy(qTs[:, tokt * 512:(tokt + 1) * 512], pqd[:])

            # ---- fused out-projection: both heads in one K=128 matmul ----
            def emit_outproj(dep, fb, flush=False):
                yns, q0 = dep
                po = psO.tile([P, 512], F32, tag="po", name="po")
                nc.tensor.matmul(po[:, :], wos[:, fb * P:(fb + 1) * P], yns[:, :],
                                 start=True, stop=True)
                oc = work.tile([P, 512], BF, tag="oc", bufs=8)
                if flush and fb % 2 == 1:
                    nc.scalar.copy(oc[:], po[:])
                else:
                    nc.vector.tensor_copy(oc[:], po[:])
                if fb % 2 == 0:
                    nc.gpsimd.dma_start(outT[fb * P:(fb + 1) * P, q0:q0 + 512], oc[:])
                else:
                    nc.sync.dma_start(outT[fb * P:(fb + 1) * P, q0:q0 + 512], oc[:])

            OP_SLOTS = {14: 0, 16: 1, 18: 2, 20: 3, 22: 4, 24: 5, 26: 6, 28: 7}

            pending = None
            for qt in range(QTS):
                q0 = qt * 512
                y0 = psY.tile([65, 512], F32, tag="y0")
                y1 = psY.tile([65, 512], F32, tag="y1")
                for kb in range(KB):
                    tgt = qt * KB + kb + LAG
                    if tgt < QTS * KB:
                        emit_s_exp(tgt // KB, tgt % KB)
                    eS = eS_store.pop((qt, kb))
                    if CLS[kb] == "A8":
                        mov0 = eS[:, 0:512].unsqueeze(1).broadcast_to([P, 2, 512])
                        mov1 = eS[:, 512:1024].unsqueeze(1).broadcast_to([P, 2, 512])
                        nc.tensor.matmul(y0[:, :], Vp80[:, kb, :, 0:65], mov0,
                                         start=(kb == 0), stop=(kb == KB - 1),
                                         perf_mode=DR)
                        nc.tensor.matmul(y1[:, :], Vp81[:, kb, :, 0:65], mov1,
                                         start=(kb == 0), stop=(kb == KB - 1),
                                         perf_mode=DR)
                    else:
                        nc.tensor.matmul(y0[:, :], VpB0[:, kb, :], eS[:, 0:512],
                                         start=(kb == 0), stop=(kb == KB - 1))
                        nc.tensor.matmul(y1[:, :], VpB1[:, kb, :], eS[:, 512:1024],
                                         start=(kb == 0), stop=(kb == KB - 1))
                    if pending is not None and kb in OP_SLOTS:
                        emit_outproj(pending, OP_SLOTS[kb])
                    if kb == 3 and qt + 1 < QTS:
                        emit_qt_proj(qt + 1)

                # ---- free y psum fast: unnormalized casts (Z rows kept in f32)
                yu0 = work.tile([64, 512], BF, tag="yu0", bufs=2)
                nc.scalar.copy(yu0[:, :], y0[0:64, :])
                yu1 = work.tile([64, 512], BF, tag="yu1", bufs=2)
                nc.scalar.copy(yu1[:, :], y1[0:64, :])
                zf = work.tile([65, 1024], F32, tag="zf", bufs=2)
                nc.vector.tensor_copy(zf[64:65, 0:512], y0[64:65, :])
                nc.vector.tensor_copy(zf[64:65, 512:1024], y1[64:65, :])

                # ---- Z -> [64,16] spread via DMA so reciprocal is parallel ----
                zsp = work.tile([64, 16], F32, tag="zsp", bufs=2)
                nc.sync.dma_start(
                    zsp[:, 0:8],
                    zf[64:65, 0:512].rearrange("o (p f) -> o p f", p=64))
                nc.sync.dma_start(
                    zsp[:, 8:16],
                    zf[64:65, 512:1024].rearrange("o (p f) -> o p f", p=64))
                zrc = work.tile([64, 16], F32, tag="zrc", bufs=2)
                nc.vector.reciprocal(zrc[:], zsp[:])
                zrw = work.tile([1, 1024], F32, tag="zrw", bufs=2)
                nc.gpsimd.dma_start(
                    zrw[0:1, 0:512].rearrange("o (p f) -> o p f", p=64),
                    zrc[:, 0:8])
                nc.gpsimd.dma_start(
                    zrw[0:1, 512:1024].rearrange("o (p f) -> o p f", p=64),
                    zrc[:, 8:16])
                zb = work.tile([64, 1024], F32, tag="zb", bufs=2)
                nc.gpsimd.partition_broadcast(zb[:, 0:512], zrw[0:1, 0:512])
                nc.gpsimd.partition_broadcast(zb[:, 512:1024], zrw[0:1, 512:1024])
                yns = work.tile([P, 512], BF, tag="yns", bufs=2)
                nc.vector.tensor_tensor(yns[0:64, :], yu0[:, :], zb[:, 0:512],
                                        op=MULT)
                y1t = work.tile([64, 512], BF, tag="y1t", bufs=2)
                nc.vector.tensor_tensor(y1t[:], yu1[:, :], zb[:, 512:1024],
                                        op=MULT)
                nc.gpsimd.dma_start(yns[64:128, :], y1t[:])
                pending = (yns, q0)

            for fb in range(CB):
                emit_outproj(pending, fb, flush=True)

    nc.compile()
    _cached = nc
    return nc


def make_in_maps(x, w_qkv, w_out):
    """x [1,T,C] f32, w_qkv [C, 3C] f32, w_out [C, C] f32 -> per-core input dicts."""
    x = np.asarray(x, dtype=np.float32)
    w_qkv = np.asarray(w_qkv, dtype=np.float32)
    w_out = np.asarray(w_out, dtype=np.float32)
    scale = 1.0 / np.sqrt(np.float32(D))
    xT = np.ascontiguousarray(x.reshape(T, C).T).astype(BF16)  # [C, T]
    in_maps = []
    for c in range(NCORES):
        cols = slice(P * c, P * (c + 1))
        wq = np.ascontiguousarray(w_qkv[:, 0:C][:, cols] * scale).astype(BF16)
        wk = np.ascontiguousarray(w_qkv[:, C:2 * C][:, cols]).astype(BF16)
        wv = np.ascontiguousarray(w_qkv[:, 2 * C:3 * C][:, cols]).astype(BF16)
        wo = np.ascontiguousarray(w_out[P * c:P * (c + 1), :]).astype(BF16)
        in_maps.append({"xT": xT, "wq": wq, "wk": wk, "wv": wv, "wo": wo})
    return in_maps


def run(x, w_qkv, w_out, trace=False):
    nc = build_bass()
    in_maps = make_in_maps(x, w_qkv, w_out)
    res = run_bass_kernel_spmd(nc, in_maps, core_ids=list(range(NCORES)), trace=trace)
    acc = np.zeros((C, T), dtype=np.float32)
    for r in res.results:
        acc += r["outT"].astype(np.float32)
    out = np.ascontiguousarray(acc.T).reshape(B, T, C)
    return out, res


def kernel(x, w_qkv, w_out):
    out, _ = run(x, w_qkv, w_out, trace=False)
    return out
